# revision 1
# baseline (speedup 1.0000x reference)
"""Trainium2 Bass kernel for multi-head dot-product GNN message passing.

Self-contained: accepts FULL inputs, shards destinations across 8 NeuronCores
internally, returns the FULL [50000, 128] output.
"""

"""Multi-head dot-product GNN message passing on TRN2 — host prep + bass builder.

Sharding: destinations are sharded across cores (each core owns NLOC nodes).
Each core processes exactly the edges whose destination is local, sorted by
destination, split into two streams by source half (dma_gather idx is int16).
Edges are packed into groups of GSZ (C subtiles of 128); each group has NW
eviction windows of WSZ edges whose destinations span < 128 local nodes.
Window partials [128 dst, 128 agg + 8 den] accumulate in PSUM via one-hot
matmuls, then dma_scatter_add them into DRAM accumulators (parity-alternated
between adjacent groups so no two in-flight scatters touch the same rows).

Per-edge math (equivalent to the reference's clamped scatter-softmax):
  attn[e,h] = exp(s)/(1 + sum_seg exp(s'))          [max-shift cancels exactly]
  out[n]    = (sum exp(s) * v[src]) / (1+den) / max(cnt,1) @ Wo.T + bo
"""

import numpy as np
import ml_dtypes

BF16 = ml_dtypes.bfloat16
SENT = 30000.0  # one-hot sentinel (never matches iota 0..127)


# ---------------------------------------------------------------------------
# Geometry
# ---------------------------------------------------------------------------
class Geom:
    def __init__(self, n_nodes, n_cores, ng, d=128, h=8, zero_bias=False):
        self.ZERO_BIAS = zero_bias
        self.N = n_nodes
        self.P = n_cores
        self.D = d
        self.H = h
        self.HD = d // h
        assert n_nodes % n_cores == 0
        self.NLOC = n_nodes // n_cores
        self.NLOC_PAD = ((self.NLOC + 127) // 128) * 128
        self.NBLK = self.NLOC_PAD // 128
        # K/V table padded to a multiple of 1024 so halves are 512-multiples
        self.N_TAB = ((n_nodes + 1023) // 1024) * 1024
        self.HALF = self.N_TAB // 2
        assert self.HALF - 1 <= 32767, "half table must fit int16"
        self.NG = ng               # groups per stream (A and B)
        self.NGRP = 2 * ng         # total groups
        self.GSZ = 1024            # edges per group (dma_gather size limit)
        self.C = 8                 # chunks (subtiles of 128) per group
        self.NW = 2                # scatter windows per group
        self.WSZ = 512             # edges per window
        self.SC_STRIDE = 256       # bf16 stride of accumulator rows (512B)
        self.SC_E = 136            # bf16 payload per row: 128 agg + 8 den
        self.ACCR = ((self.NLOC_PAD + 128 + 511) // 512) * 512
        self.QROWS = ((self.NLOC_PAD + 511) // 512) * 512


# ---------------------------------------------------------------------------
# Host-side edge packing
# ---------------------------------------------------------------------------
def pack_core(g: Geom, src, dst, core):
    """Pack one core's edges into the group/window structure."""
    lo = core * g.NLOC
    m = (dst >= lo) & (dst < lo + g.NLOC)
    s, d = src[m].astype(np.int64), (dst[m] - lo).astype(np.int64)

    cnt = np.bincount(d, minlength=g.NLOC_PAD).astype(np.float32)
    cnt_t = np.maximum(cnt, 1.0).reshape(g.NBLK, 128).T.copy()  # [128, NBLK]

    kvidx = np.zeros((128, g.NGRP, g.GSZ // 16), np.int16)
    qidx = np.zeros((128, g.NGRP, g.GSZ // 16), np.int16)
    dstrel = np.full((128, g.NGRP * g.C), SENT, BF16)
    scidx = np.zeros((128, g.NGRP, g.NW * 128 // 16), np.int16)
    trash = g.ACCR - 128  # rows whose scatter payload is always zero
    for grp in range(g.NGRP):  # default scatter rows: trash (adds zeros)
        for jj in range(g.NW * 128):
            scidx[jj % 16, grp, jj // 16] = trash + jj % 128

    for half in (0, 1):
        hm = (s >= g.HALF) == bool(half)
        hs = (s[hm] - half * g.HALF).astype(np.int64)
        hd = d[hm]
        order = np.argsort(hd, kind="stable")
        hs, hd = hs[order], hd[order]
        n = len(hd)
        # windows: up to WSZ edges, dst span < 128, cut at COMPLETE dst
        # boundaries so no two windows' live rows overlap (scatter-add RMW
        # from different SDMA engines would race on shared rows)
        wins = []
        i = 0
        while i < n:
            base = hd[i]
            j = i
            while j < n and j - i < g.WSZ and hd[j] < base + 128:
                j += 1
            if j < n and j > i and hd[j] == hd[j - 1]:
                jc = j
                while jc > i and hd[jc - 1] == hd[j]:
                    jc -= 1
                if jc > i:  # back up to keep the straddling dst whole
                    j = jc
            wins.append((int(base), hs[i:j], hd[i:j] - base))
            i = j
        n_groups = (len(wins) + g.NW - 1) // g.NW
        assert n_groups <= g.NG, (
            f"core {core} half {half}: need {n_groups} groups > NG={g.NG}"
        )
        for w, (base, ws, wrel) in enumerate(wins):
            grp = half * g.NG + w // g.NW
            wig = w % g.NW  # window index within group
            lastrel = int(wrel[-1]) if len(ws) else -1
            for jj in range(128):
                sj = wig * 128 + jj
                scidx[sj % 16, grp, sj // 16] = (
                    base + jj if jj <= lastrel else trash + jj
                )
            for k in range(len(ws)):
                j = wig * g.WSZ + k  # slot within group
                kvidx[j % 16, grp, j // 16] = ws[k]
                qidx[j % 16, grp, j // 16] = base + wrel[k]  # local dst
                dstrel[j % 128, grp * g.C + j // 128] = float(wrel[k])

    for arr in (kvidx, qidx, scidx):  # ucode reads idxs replicated per 16-row stripe
        for k in range(1, 8):
            arr[16 * k : 16 * (k + 1)] = arr[0:16]
    return dict(kvidx=kvidx, qidx=qidx, dstrel=dstrel, scidx=scidx, cnt_t=cnt_t)


def host_prep(g: Geom, feats, edge_index, Wq, bq, Wk, bk, Wv, bv, Wo, bo):
    """Build per-core input maps (list of dicts name->np.ndarray)."""
    src = np.asarray(edge_index[:, 0], np.int64)
    dst = np.asarray(edge_index[:, 1], np.int64)
    feats = np.asarray(feats, np.float32)

    feats_pad = np.zeros((g.N_TAB, g.D), np.float32)
    feats_pad[: g.N] = feats
    featsT = np.ascontiguousarray(feats_pad.T)

    iota_row = np.tile(np.arange(128, dtype=np.float32)[None, :], (128, 1))
    ident = np.eye(128, dtype=np.float32)
    ones_row = np.ones((1, 128), np.float32)

    common = dict(
        featsT=featsT.astype(BF16),
        WqT=np.ascontiguousarray(Wq.T.astype(BF16)),
        WkT=np.ascontiguousarray(Wk.T.astype(BF16)),
        WvT=np.ascontiguousarray(Wv.T.astype(BF16)),
        WoT=np.ascontiguousarray(Wo.T.astype(np.float32)),
        bq=bq.astype(BF16).reshape(1, g.D),
        bk=bk.astype(BF16).reshape(1, g.D),
        bv=bv.astype(BF16).reshape(1, g.D),
        bo=bo.astype(np.float32).reshape(1, g.D),
        iota_row=iota_row.astype(BF16),
        ident=ident,
        ones_row=ones_row,
        ones_bf=ones_row.astype(BF16),
    )

    maps = []
    for c in range(g.P):
        featsL = np.zeros((g.QROWS, g.D), np.float32)
        featsL[: g.NLOC] = feats[c * g.NLOC : (c + 1) * g.NLOC]
        mc = dict(common)
        mc["featsLT"] = np.ascontiguousarray(featsL.T.astype(BF16))
        mc.update(pack_core(g, src, dst, c))
        maps.append(mc)
    return maps


# ---------------------------------------------------------------------------
# Numpy golden model of the DEVICE algorithm (validates pack_core + math)
# ---------------------------------------------------------------------------
def golden_core(g: Geom, m):
    f32a = lambda x: np.asarray(x, np.float32)
    feats = f32a(m["featsT"]).T
    K = (feats @ f32a(m["WkT"]) + f32a(m["bk"])).astype(BF16).astype(np.float32)
    V = (feats @ f32a(m["WvT"]) + f32a(m["bv"])).astype(BF16).astype(np.float32)
    Q = (f32a(m["featsLT"]).T @ f32a(m["WqT"]) + f32a(m["bq"])).astype(BF16).astype(np.float32)

    acc = [np.zeros((g.ACCR, g.SC_STRIDE), np.float32) for _ in range(2)]

    for grp in range(g.NGRP):
        half = grp // g.NG
        base_tab = half * g.HALF
        kv_i = np.array(
            [m["kvidx"][j % 16, grp, j // 16] for j in range(g.GSZ)], np.int64
        )
        q_i = np.array(
            [m["qidx"][j % 16, grp, j // 16] for j in range(g.GSZ)], np.int64
        )
        rel = np.array(
            [float(m["dstrel"][j % 128, grp * g.C + j // 128]) for j in range(g.GSZ)]
        )
        sc_i = np.array(
            [m["scidx"][j % 16, grp, j // 16] for j in range(g.NW * 128)], np.int64
        )
        kg = K[base_tab + kv_i]
        vg = V[base_tab + kv_i]
        qg = Q[q_i]
        prod = (qg * kg).reshape(g.GSZ, g.H, g.HD)
        w = np.exp(0.25 * prod.sum(-1))
        wv = (w[:, :, None] * vg.reshape(g.GSZ, g.H, g.HD)).reshape(g.GSZ, g.D)
        oh = (rel[:, None] == np.arange(128)[None, :]).astype(np.float32)
        a = acc[grp % 2]
        for win in range(g.NW):
            sl = slice(win * g.WSZ, (win + 1) * g.WSZ)
            pagg = oh[sl].T @ wv[sl]     # [128 dst, 128]
            pden = oh[sl].T @ w[sl]      # [128 dst, 8]
            rows = sc_i[win * 128 : (win + 1) * 128]
            a[rows, 0:128] += pagg
            a[rows, 128:136] += pden

    asum = acc[0] + acc[1]
    den = asum[: g.NLOC_PAD, 128:136]
    agg = asum[: g.NLOC_PAD, 0:128]
    cnt = m["cnt_t"].T.reshape(-1)[: g.NLOC_PAD]
    fac = 1.0 / ((den + 1.0) * cnt[:, None])
    agf = (agg.reshape(-1, g.H, g.HD) * fac[:, :, None]).reshape(-1, g.D)
    out = agf @ m["WoT"] + m["bo"]       # [NLOC_PAD, 128]
    return np.ascontiguousarray(out.T)   # [128, NLOC_PAD]


def golden_full(g: Geom, maps):
    outs = [golden_core(g, m) for m in maps]
    return np.concatenate([o[:, : g.NLOC].T for o in outs], axis=0)


# ---------------------------------------------------------------------------
# Bass program
# ---------------------------------------------------------------------------
def build_bass(g: Geom):
    import os
    from contextlib import ExitStack

    import concourse.bass as bass
    import concourse.bacc as bacc
    import concourse.mybir as mybir
    import concourse.tile as tile
    from concourse.library_config import mlp

    f32 = mybir.dt.float32
    bf = mybir.dt.bfloat16
    i16 = mybir.dt.int16
    AL = mybir.AluOpType
    ACT = mybir.ActivationFunctionType

    nc = bacc.Bacc("TRN2", target_bir_lowering=False, num_devices=g.P)

    # --- I/O -------------------------------------------------------------
    featsT = nc.dram_tensor("featsT", [128, g.N_TAB], bf, kind="ExternalInput")
    featsLT = nc.dram_tensor("featsLT", [128, g.QROWS], bf, kind="ExternalInput")
    wts = {
        n: nc.dram_tensor(n, [g.D, g.D], f32 if n == "WoT" else bf,
                          kind="ExternalInput")
        for n in ("WqT", "WkT", "WvT", "WoT")
    }
    bias = {
        n: nc.dram_tensor(n, [1, g.D], f32 if n == "bo" else bf,
                          kind="ExternalInput")
        for n in ("bq", "bk", "bv", "bo")
    }
    kvidx_d = nc.dram_tensor(
        "kvidx", [128, g.NGRP, g.GSZ // 16], i16, kind="ExternalInput"
    )
    qidx_d = nc.dram_tensor(
        "qidx", [128, g.NGRP, g.GSZ // 16], i16, kind="ExternalInput"
    )
    dstrel_d = nc.dram_tensor(
        "dstrel", [128, g.NGRP * g.C], bf, kind="ExternalInput"
    )
    scidx_d = nc.dram_tensor(
        "scidx", [128, g.NGRP, g.NW * 8], i16, kind="ExternalInput"
    )
    cnt_d = nc.dram_tensor("cnt_t", [128, g.NBLK], f32, kind="ExternalInput")
    iota_d = nc.dram_tensor("iota_row", [128, 128], bf, kind="ExternalInput")
    ident_d = nc.dram_tensor("ident", [128, 128], f32, kind="ExternalInput")
    ones_d = nc.dram_tensor("ones_row", [1, 128], f32, kind="ExternalInput")
    onesbf_d = nc.dram_tensor("ones_bf", [1, 128], bf, kind="ExternalInput")

    outT = nc.dram_tensor("outT", [128, g.NLOC_PAD], f32, kind="ExternalOutput")
    # scatter accumulators (bf16: each row gets at most one add per
    # stream, so RMW rounding is bounded), zeroed on-device before phase 2
    acc_d = [
        nc.dram_tensor(f"acc{i}", [g.ACCR, g.SC_STRIDE], bf)
        for i in range(2)
    ]

    # --- DRAM scratch ----------------------------------------------------
    KV_h = [
        nc.dram_tensor(f"KV_tab{i}", [g.HALF, 2 * g.D], bf) for i in range(2)
    ]
    Q_t = nc.dram_tensor("Q_tab", [g.QROWS, g.D], bf)

    NCH = g.N_TAB // 512
    NCHQ = g.QROWS // 512

    with tile.TileContext(nc) as tc, ExitStack() as ctx:
        nc.gpsimd.load_library(mlp)

        # pre-allocated count registers: to_reg(int) per gather call would
        # leak one Pool register per call and exhaust the register file
        sv_gsz = nc.alloc_register(mybir.EngineType.Pool, "rgsz")
        nc.gpsimd.reg_mov(sv_gsz, g.GSZ)
        sv_scn2 = nc.alloc_register(mybir.EngineType.Pool, "rscn2")
        nc.gpsimd.reg_mov(sv_scn2, 2 * g.NW * 128)

        const = ctx.enter_context(tc.tile_pool(name="const", bufs=1))
        w_t = {
            n: const.tile([g.D, g.D], f32 if n == "WoT" else bf, tag=n, name=n + "_t")
            for n in wts
        }
        for n in wts:
            nc.sync.dma_start(w_t[n][:], wts[n][:])
        b_t = {
            n: const.tile([1, g.D], f32 if n == "bo" else bf, tag=n, name=n + "_t")
            for n in bias
        }
        for n in bias:
            nc.sync.dma_start(b_t[n][:], bias[n][:])
        iota_t = const.tile([128, 128], bf, tag="iota")
        nc.sync.dma_start(iota_t[:], iota_d[:])
        id_t = const.tile([128, 128], f32, tag="ident")
        nc.sync.dma_start(id_t[:], ident_d[:])
        ones_t = const.tile([1, 128], f32, tag="ones")
        nc.sync.dma_start(ones_t[:], ones_d[:])
        onesbf_t = const.tile([1, 128], bf, tag="onesbf")
        nc.sync.dma_start(onesbf_t[:], onesbf_d[:])
        kvidx_t = const.tile([128, g.NGRP, g.GSZ // 16], i16, tag="kvidx")
        nc.sync.dma_start(kvidx_t[:], kvidx_d[:])
        qidx_t = const.tile([128, g.NGRP, g.GSZ // 16], i16, tag="qidx")
        nc.sync.dma_start(qidx_t[:], qidx_d[:])
        dstrel_t = const.tile([128, g.NGRP * g.C], bf, tag="dstrel")
        nc.sync.dma_start(dstrel_t[:], dstrel_d[:])
        scidx_t = const.tile([128, g.NGRP, g.NW * 8], i16, tag="scidx")
        nc.sync.dma_start(scidx_t[:], scidx_d[:])
        cnt_t = const.tile([128, g.NBLK], f32, tag="cnt")
        nc.sync.dma_start(cnt_t[:], cnt_d[:])

        # zero the scatter accumulators (DRAM contents are undefined)
        with tc.tile_pool(name="zp", bufs=1) as zp:
            zt = zp.tile([128, 4 * g.SC_STRIDE], bf, tag="zt", name="zt")
            nc.vector.memset(zt[:], 0.0)
            zview = [
                a[:].rearrange("(r p) e -> p r e", p=128) for a in acc_d
            ]
            for a in ([] if os.environ.get("SKIP_ZERO") == "1" else zview):
                for r in range(g.ACCR // 512):
                    nc.sync.dma_start(
                        a[:, 4 * r : 4 * (r + 1), :],
                        zt[:].rearrange("p (c e) -> p c e", c=4),
                    )

        # ---------------- Phase 1: projections --------------------------
        with (
            tc.tile_pool(name="p1", bufs=6) as p1,
            tc.tile_pool(name="p1ps", bufs=2, space="PSUM") as p1ps,
        ):
            def proj_chunk(srcT_dram, ci, tabs, copy_engines):
                # one combined [k|v] row image in SBUF -> single contiguous
                # row DMA (512B runs) instead of two strided half-row DMAs
                ftT = p1.tile([128, 512], bf, tag="ftT", name="ftT")
                nc.sync.dma_start(ftT[:], srcT_dram[:, 512 * ci : 512 * (ci + 1)])
                nslots = len(tabs)
                cp = p1.tile([128, 4, nslots, 128], bf, tag=f"cp{nslots}",
                             name=f"cp{nslots}")
                for slot, ((wn, bn, tab), ceng) in enumerate(
                    zip(tabs, copy_engines)
                ):
                    ps = p1ps.tile([128, 4, 128], f32, tag="ps" + wn, name="ps" + wn)
                    for j in range(4):
                        if not g.ZERO_BIAS:
                            nc.tensor.matmul(
                                ps[:, j, :], onesbf_t[:], b_t[bn][:],
                                start=True, stop=False,
                            )
                        nc.tensor.matmul(
                            ps[:, j, :], ftT[:, 128 * j : 128 * (j + 1)], w_t[wn][:],
                            start=g.ZERO_BIAS, stop=True,
                        )
                    if ceng == "act":
                        nc.scalar.activation(cp[:, :, slot, :], ps[:], ACT.Copy)
                    else:
                        nc.vector.tensor_copy(cp[:, :, slot, :], ps[:])
                for slot, (wn, bn, tab) in enumerate(tabs):
                    pass
                tabs[0][2](ci, cp)

            _skip_p1 = os.environ.get("SKIP_P1") == "1"
            KV_rows = [
                t[:].rearrange("(c p) e -> p c e", p=128) for t in KV_h
            ]
            Q_rows = Q_t[:].rearrange("(c p) d -> p c d", p=128)
            NCHH = NCH // 2  # chunks per table half

            def wr_kv(ci, cp):
                half, cih = divmod(ci, NCHH)
                nc.sync.dma_start(
                    KV_rows[half][:, 4 * cih : 4 * (cih + 1), :],
                    cp[:].rearrange("p c s d -> p c (s d)"),
                )

            def wr_q(ci, cp):
                nc.sync.dma_start(
                    Q_rows[:, 4 * ci : 4 * (ci + 1), :],
                    cp[:].rearrange("p c s d -> p c (s d)"),
                )

            # Q first (gates every edge group), then KV half A (gates the
            # A-stream groups), then KV half B — so B-half projection DMA
            # overlaps A-stream edge processing.
            for ci in range(0 if _skip_p1 else NCHQ):
                proj_chunk(featsLT, ci, [("WqT", "bq", wr_q)], ["act"])
            for ci in range(0 if _skip_p1 else NCH):
                proj_chunk(
                    featsT, ci,
                    [("WkT", "bk", wr_kv), ("WvT", "bv", None)],
                    ["act", "dve"],
                )

        # ---------------- Phase 2: edges ---------------------------------
        with (
            tc.tile_pool(name="gat", bufs=3) as gat,
            tc.tile_pool(name="ew", bufs=3) as ew,
            tc.tile_pool(name="eps", bufs=3, space="PSUM") as eps,
        ):
            for grp in range(g.NGRP):
                tab_KV = KV_h[0][:] if grp < g.NG else KV_h[1][:]
                kvi = kvidx_t[:, grp, :]
                qi = qidx_t[:, grp, :]

                kvg = gat.tile([128, g.C, 2, 128], bf, tag="kvg", name="kvg")
                nc.gpsimd.dma_gather(
                    kvg[:].rearrange("p c two d -> p c (two d)"),
                    tab_KV, kvi, g.GSZ, sv_gsz, 2 * g.D, queue_num=0,
                )
                kg = kvg[:, :, 0, :]
                vg = kvg[:, :, 1, :]
                qg = gat.tile([128, g.C, 128], bf, tag="qg", name="qg")
                nc.gpsimd.dma_gather(qg[:], Q_t[:, :], qi, g.GSZ, sv_gsz, 128, queue_num=0)

                prod = ew.tile([128, g.C, 128], bf, tag="prod", name="prod")
                nc.vector.tensor_tensor(prod[:], qg[:], kg, AL.mult)
                sc = ew.tile([128, g.C, g.H], f32, tag="sc", name="sc")
                nc.vector.tensor_reduce(
                    sc[:],
                    prod[:].rearrange("p c (h d) -> p c h d", d=g.HD),
                    mybir.AxisListType.X,
                    AL.add,
                )
                wexp = ew.tile([128, g.C, g.H], bf, tag="wexp", name="wexp")
                nc.scalar.activation(wexp[:], sc[:], ACT.Exp, scale=0.25)
                wv = ew.tile([128, g.C, 128], bf, tag="wv", name="wv")
                nc.vector.tensor_tensor(
                    wv[:].rearrange("p c (h d) -> p c h d", d=g.HD),
                    vg.rearrange("p c (h d) -> p c h d", d=g.HD),
                    wexp[:].broadcast_to([128, g.C, g.H, g.HD]),
                    AL.mult,
                )
                oh = ew.tile([128, g.C, 128], bf, tag="oh", name="oh")
                nc.vector.tensor_tensor(
                    oh[:],
                    dstrel_t[:, grp * g.C : (grp + 1) * g.C].broadcast_to(
                        [128, g.C, 128]
                    ),
                    iota_t[:]
                    .rearrange("p (c j) -> p c j", c=1)
                    .broadcast_to([128, g.C, 128]),
                    AL.is_equal,
                )

                if grp % 2 == 0:
                    stg2 = ew.tile(
                        [128, 2, g.NW, g.SC_E], bf, tag="stg2", name="stg2"
                    )
                stg = stg2[:, grp % 2]
                for win in range(g.NW):
                    pa = eps.tile([128, 128], f32, tag="pagg", name="pagg")
                    pd = eps.tile([128, g.H], f32, tag="pden", name="pden")
                    s0 = win * (g.C // g.NW)
                    s1 = s0 + g.C // g.NW
                    for s in range(s0, s1):
                        nc.tensor.matmul(
                            pa[:], oh[:, s, :], wv[:, s, :],
                            start=(s == s0), stop=(s == s1 - 1),
                        )
                        nc.tensor.matmul(
                            pd[:], oh[:, s, :], wexp[:, s, :],
                            start=(s == s0), stop=(s == s1 - 1),
                        )
                    nc.scalar.activation(stg[:, win, 0:128], pa[:], ACT.Copy)
                    nc.scalar.activation(stg[:, win, 128 : g.SC_E], pd[:], ACT.Copy)

                if grp % 2 == 1:
                    nc.gpsimd.dma_scatter_add(
                        acc_d[(grp // 2) % 2][:, 0 : g.SC_E],
                        stg2[:].rearrange("p a w e -> p (a w) e"),
                        scidx_t[:, grp - 1 : grp + 1, :].rearrange(
                            "p t s -> p (t s)"
                        ),
                        2 * g.NW * 128,
                        sv_scn2,
                        g.SC_E,
                        elem_step=g.SC_STRIDE,
                        queue_num=0,
                    )

        # ---------------- Phase 3: finalize ------------------------------
        with (
            tc.tile_pool(name="fin", bufs=4) as fin,
            tc.tile_pool(name="fps", bufs=3, space="PSUM") as fps,
            tc.tile_pool(name="fps2", bufs=3, space="PSUM") as fps2,
        ):
            def fin_batch(b0, nb):
                rows = slice(b0 * 128, (b0 + nb) * 128)
                a0 = fin.tile([128, nb, g.SC_E], bf, tag="a0", name="a0")
                nc.sync.dma_start(
                    a0[:], acc_d[0][:].rearrange("(r p) e -> p r e", p=128)[
                        :, b0 * 1 : b0 + nb, 0 : g.SC_E
                    ] if False else
                    acc_d[0][:].rearrange("(r p) e -> p r e", p=128)[
                        :, b0 : b0 + nb, 0 : g.SC_E
                    ],
                )
                a1 = fin.tile([128, nb, g.SC_E], bf, tag="a1", name="a1")
                nc.sync.dma_start(
                    a1[:],
                    acc_d[1][:].rearrange("(r p) e -> p r e", p=128)[
                        :, b0 : b0 + nb, 0 : g.SC_E
                    ],
                )
                asum = fin.tile([128, nb, g.SC_E], f32, tag="asum", name="asum")
                nc.vector.tensor_tensor(asum[:], a0[:], a1[:], AL.add)
                dent = fin.tile([128, nb, g.H], f32, tag="dent", name="dent")
                nc.vector.scalar_tensor_tensor(
                    dent[:],
                    asum[:, :, 128 : g.SC_E],
                    1.0,
                    cnt_t[:, b0 : b0 + nb]
                    .rearrange("p r -> p r")
                    .broadcast_to([128, nb, g.H]),
                    AL.add,
                    AL.mult,
                )
                fac = fin.tile([128, nb, g.H], f32, tag="fac", name="fac")
                nc.vector.reciprocal(fac[:], dent[:])
                agf = fin.tile([128, nb, 128], f32, tag="agf", name="agf")
                nc.vector.tensor_tensor(
                    agf[:].rearrange("p r (h d) -> p r h d", d=g.HD),
                    asum[:, :, 0:128].rearrange("p r (h d) -> p r h d", d=g.HD),
                    fac[:].broadcast_to([128, nb, g.H, g.HD]),
                    AL.mult,
                )
                pt = fps.tile([128, nb, 128], f32, tag="pt", name="pt")
                for j in range(nb):
                    nc.tensor.transpose(pt[:, j, :], agf[:, j, :], id_t[:])
                agfT = fin.tile([128, nb, 128], f32, tag="agfT", name="agfT")
                nc.scalar.activation(agfT[:], pt[:], ACT.Copy)
                po = fps2.tile([128, nb, 128], f32, tag="po", name="po")
                for j in range(nb):
                    nc.tensor.matmul(
                        po[:, j, :], b_t["bo"][:], ones_t[:],
                        start=True, stop=False,
                    )
                    nc.tensor.matmul(
                        po[:, j, :], w_t["WoT"][:], agfT[:, j, :],
                        start=False, stop=True,
                    )
                oc = fin.tile([128, nb, 128], f32, tag="oc", name="oc")
                nc.scalar.activation(oc[:], po[:], ACT.Copy)
                nc.sync.dma_start(
                    outT[:].rearrange("p (r d) -> p r d", d=128)[:, b0 : b0 + nb, :],
                    oc[:],
                )

            if os.environ.get("SKIP_P3") != "1":
                b0 = 0
                while b0 < g.NBLK:
                    nb = min(4, g.NBLK - b0)
                    fin_batch(b0, nb)
                    b0 += nb

    nc.compile()
    return nc


# ---------------------------------------------------------------------------
# Entry point
# ---------------------------------------------------------------------------
N_NODES = 50000
N_CORES = 8

_CACHE = {}


def _needed_ng(g, src, dst):
    need = 1
    for core in range(g.P):
        lo = core * g.NLOC
        m = (dst >= lo) & (dst < lo + g.NLOC)
        s, d = src[m], dst[m] - lo
        for half in (0, 1):
            hm = (s >= g.HALF) == bool(half)
            hd = np.sort(d[hm], kind="stable")
            n = len(hd)
            wins = 0
            i = 0
            while i < n:
                base = hd[i]
                j = i
                while j < n and j - i < g.WSZ and hd[j] < base + 128:
                    j += 1
                wins += 1
                i = j
            need = max(need, (wins + g.NW - 1) // g.NW)
    return need


def kernel(**inputs):
    from concourse.bass_utils import run_bass_kernel_spmd

    feats = np.asarray(inputs["feats"], np.float32)
    edge_index = np.asarray(inputs["edge_index"], np.int64)
    src = edge_index[:, 0]
    dst = edge_index[:, 1]

    zb = all(
        not np.any(np.asarray(inputs[k]))
        for k in ("bq", "bk", "bv")
    )
    g0 = Geom(N_NODES, N_CORES, ng=1)
    ng = _needed_ng(g0, src, dst)
    g = Geom(N_NODES, N_CORES, ng=ng, zero_bias=zb)

    maps = host_prep(
        g, feats, edge_index,
        np.asarray(inputs["Wq"], np.float32), np.asarray(inputs["bq"], np.float32),
        np.asarray(inputs["Wk"], np.float32), np.asarray(inputs["bk"], np.float32),
        np.asarray(inputs["Wv"], np.float32), np.asarray(inputs["bv"], np.float32),
        np.asarray(inputs["Wo"], np.float32), np.asarray(inputs["bo"], np.float32),
    )

    key = (ng, zb)
    if key not in _CACHE:
        _CACHE[key] = build_bass(g)
    nc = _CACHE[key]

    res = run_bass_kernel_spmd(nc, maps, list(range(N_CORES)))
    out = np.empty((N_NODES, g.D), np.float32)
    for c in range(N_CORES):
        out[c * g.NLOC : (c + 1) * g.NLOC] = res.results[c]["outT"][:, : g.NLOC].T
    return out



# revision 3
# speedup vs baseline: 1.1011x; 1.1011x over previous
"""Trainium2 Bass kernel v3: multi-head dot-product GNN message passing.

Self-contained: accepts FULL inputs, shards destinations across 8 NeuronCores,
returns the FULL [50000, 128] output.

Design (block-resident partials, no DRAM accumulator):
- Destinations sharded across cores (NLOC each); each core's edges grouped by
  128-aligned dst BLOCK, split into two streams by source half (gather idx is
  int16, table has 50176 rows). Per (block, stream) the edge count is padded to
  a multiple of 128; the static per-block schedule (SA[b], SB[b]) is the max
  over cores so one program serves all cores (SPMD). Blocks are processed in
  PAIRS sharing one gather per stream (slot layout: A[b0] A[b1] B[b0] B[b1]).
- Per block: build per-subtile one-hot oh[e,d] (tensor_scalar 4x), PE-transpose
  to ohT[d,e], expand per-edge Q via Qe = ohT^T @ Qblk on the PE (Q lives in
  SBUF, node-major, 128-aligned blocks); scores via 2x TT multiply +
  binary-tree head reduce; exp on ACT; V is stored hd-major so the exp-weight
  broadcast keeps innermost packing (2x TT); aggregate [pa|pd] in PSUM via one
  one-hot matmul per subtile (partials are FINAL: every dst lives in exactly
  one block); finalize in place and write the output block. No scatter-add, no
  accumulator zero/readback, no per-edge Q gather.

Per-edge math (identical to reference's clamped scatter-softmax):
  attn[e,h] = exp(s)/(1 + sum_seg exp(s'))      [max-shift cancels exactly]
  out[n]    = (sum exp(s) * v[src]) / (1+den) / max(cnt,1) @ Wo.T + bo
"""

import numpy as np
import ml_dtypes

BF16 = ml_dtypes.bfloat16
SENT = 30000.0  # one-hot sentinel (never matches iota 0..127)

# V/Wo head-dim-major permutation: col j=(hd*8+h) <- col h*16+hd
PERM = np.array([(j % 8) * 16 + j // 8 for j in range(128)], np.int64)


# ---------------------------------------------------------------------------
# Geometry + static schedule
# ---------------------------------------------------------------------------
class Geom:
    def __init__(self, n_nodes, n_cores, bases, sched_a, sched_b, d=128, h=8,
                 zero_bias=False):
        self.ZERO_BIAS = zero_bias
        self.N = n_nodes
        self.P = n_cores
        self.D = d
        self.H = h
        self.HD = d // h
        assert n_nodes % n_cores == 0
        self.NLOC = n_nodes // n_cores
        self.NLOC_PAD = ((self.NLOC + 127) // 128) * 128
        self.N_TAB = ((n_nodes + 1023) // 1024) * 1024
        self.HALF = self.N_TAB // 2
        assert self.HALF - 1 <= 32767
        self.QROWS = ((self.NLOC_PAD + 511) // 512) * 512
        self.BASES = tuple(int(x) for x in bases)   # shared block bases
        self.NBLK = len(self.BASES)
        self.WIDTHS = tuple(
            (self.BASES[i + 1] if i + 1 < self.NBLK else self.NLOC)
            - self.BASES[i]
            for i in range(self.NBLK)
        )
        assert all(0 < w <= 128 for w in self.WIDTHS)
        self.SA = tuple(int(x) for x in sched_a)
        self.SB = tuple(int(x) for x in sched_b)
        assert len(self.SA) == self.NBLK and len(self.SB) == self.NBLK
        assert max(self.SA) <= 8 and max(self.SB) <= 8  # gather <= 1024 idx
        self.S = tuple(a + b for a, b in zip(self.SA, self.SB))
        self.SMAX = max(self.S)
        # pair-grouped slot layout: for pair (b0, b1): A[b0] A[b1] B[b0] B[b1]
        self.PAIRS = []
        astart = [0] * self.NBLK
        bstart = [0] * self.NBLK
        off = 0
        b = 0
        while b < self.NBLK:
            blks = [b] if b + 1 >= self.NBLK else [b, b + 1]
            ga_start = off
            for bb in blks:
                astart[bb] = off
                off += self.SA[bb]
            gb_start = off
            for bb in blks:
                bstart[bb] = off
                off += self.SB[bb]
            self.PAIRS.append(
                (blks, ga_start, gb_start - ga_start, gb_start, off - gb_start)
            )
            b += 2
        self.ASTART = tuple(astart)
        self.BSTART = tuple(bstart)
        self.TOTSUB = off
        self.CH = 4  # subtile chunk size (PSUM staging granularity)


def compute_schedule(n_nodes, n_cores, src, dst, cap=1024):
    """Shared variable-width block cuts: every (core, block, stream) count
    <= cap so each stream fits one dma_gather call."""
    NLOC = n_nodes // n_cores
    N_TAB = ((n_nodes + 1023) // 1024) * 1024
    HALF = N_TAB // 2
    degA = np.zeros((n_cores, NLOC), np.int64)
    degB = np.zeros((n_cores, NLOC), np.int64)
    for c in range(n_cores):
        lo = c * NLOC
        m = (dst >= lo) & (dst < lo + NLOC)
        s, d = src[m], dst[m] - lo
        hB = s >= HALF
        np.add.at(degA[c], d[~hB], 1)
        np.add.at(degB[c], d[hB], 1)
    bases, base, accA, accB = [0], 0, np.zeros(n_cores, np.int64), np.zeros(
        n_cores, np.int64)
    nA_blocks, nB_blocks = [], []
    for n in range(NLOC):
        w = n - base
        if (w >= 128 or (accA + degA[:, n]).max() > cap
                or (accB + degB[:, n]).max() > cap):
            nA_blocks.append(accA.copy())
            nB_blocks.append(accB.copy())
            bases.append(n)
            base = n
            accA[:] = 0
            accB[:] = 0
        accA += degA[:, n]
        accB += degB[:, n]
    nA_blocks.append(accA.copy())
    nB_blocks.append(accB.copy())
    SA = [max(1, int(np.ceil(a.max() / 128))) for a in nA_blocks]
    SB = [max(1, int(np.ceil(b.max() / 128))) for b in nB_blocks]
    return tuple(bases), tuple(SA), tuple(SB)


# ---------------------------------------------------------------------------
# Host-side packing
# ---------------------------------------------------------------------------
def pack_core(g: Geom, src, dst, core):
    """Per-core kvidx [128, TOTSUB*8] i16 and dstrel [128, TOTSUB] f32."""
    lo = core * g.NLOC
    m = (dst >= lo) & (dst < lo + g.NLOC)
    s, d = src[m].astype(np.int64), (dst[m] - lo).astype(np.int64)
    blk = np.searchsorted(np.array(g.BASES), d, side="right") - 1
    rel = d - np.array(g.BASES)[blk]

    cnt = np.bincount(d, minlength=g.NLOC).astype(np.float32)
    cm = np.ones((128, g.NBLK), np.float32)
    for b in range(g.NBLK):
        w = g.WIDTHS[b]
        cm[:w, b] = np.maximum(cnt[g.BASES[b] : g.BASES[b] + w], 1.0)

    kvidx = np.zeros((g.TOTSUB * 128,), np.int16)
    dstrel = np.full((g.TOTSUB * 128,), SENT, np.float32)

    for b in range(g.NBLK):
        mb = blk == b
        sb_, rb_ = s[mb], rel[mb]
        hB = sb_ >= g.HALF
        for half, (ss, rr) in enumerate(
            ((sb_[~hB], rb_[~hB]), (sb_[hB] - g.HALF, rb_[hB]))
        ):
            off = g.ASTART[b] if half == 0 else g.BSTART[b]
            nslot = (g.SA[b] if half == 0 else g.SB[b]) * 128
            n = len(ss)
            assert n <= nslot, (core, b, half, n, nslot)
            base = off * 128
            kvidx[base : base + n] = ss.astype(np.int16)
            dstrel[base : base + n] = rr.astype(np.float32)

    kvw = np.zeros((128, g.TOTSUB * 8), np.int16)
    kvw[0:16] = kvidx.reshape(-1, 16).T
    for k in range(1, 8):
        kvw[16 * k : 16 * (k + 1)] = kvw[0:16]
    drl = dstrel.reshape(g.TOTSUB, 128).T.astype(np.float32).copy()
    return dict(kvidx=kvw, dstrel=drl,
                dstrel_row=dstrel.astype(BF16).reshape(1, -1), cm_t=cm)


def host_prep(g: Geom, feats, edge_index, Wq, bq, Wk, bk, Wv, bv, Wo, bo):
    src = np.asarray(edge_index[:, 0], np.int64)
    dst = np.asarray(edge_index[:, 1], np.int64)
    feats = np.asarray(feats, np.float32)

    feats_pad = np.zeros((g.N_TAB, g.D), np.float32)
    feats_pad[: g.N] = feats
    featsT = np.ascontiguousarray(feats_pad.T)

    iota_row = np.tile(np.arange(128, dtype=np.float32)[None, :], (128, 1))

    WvTp = np.ascontiguousarray(Wv.T[:, PERM])   # V output cols hd-major
    WoTp = np.ascontiguousarray(Wo.T[PERM, :])   # Wo input rows hd-major

    common = dict(
        featsT=featsT.astype(BF16),
        WqT=np.ascontiguousarray(Wq.T.astype(BF16)),
        WkT=np.ascontiguousarray(Wk.T.astype(BF16)),
        WvT=WvTp.astype(BF16),
        WoT=WoTp.astype(np.float32),
        bq=bq.astype(BF16).reshape(1, g.D),
        bk=bk.astype(BF16).reshape(1, g.D),
        bv=bv[PERM].astype(BF16).reshape(1, g.D),
        bo=bo.astype(np.float32).reshape(1, g.D),
        iota_row=iota_row.astype(BF16),
        ident=np.eye(128, dtype=np.float32),
        ident_bf=np.eye(128, dtype=np.float32).astype(BF16),
        ones_row=np.ones((1, 128), np.float32),
        ones_bf=np.ones((1, 128), np.float32).astype(BF16),
    )

    maps = []
    for c in range(g.P):
        featsL = np.zeros((g.QROWS, g.D), np.float32)
        featsL[: g.NLOC] = feats[c * g.NLOC : (c + 1) * g.NLOC]
        mc = dict(common)
        mc["featsLT"] = np.ascontiguousarray(featsL.T.astype(BF16))
        mc.update(pack_core(g, src, dst, c))
        maps.append(mc)
    return maps


# ---------------------------------------------------------------------------
# Numpy golden model of the DEVICE algorithm
# ---------------------------------------------------------------------------
def golden_core(g: Geom, m):
    f32a = lambda x: np.asarray(x, np.float32)
    feats = f32a(m["featsT"]).T
    K = (feats @ f32a(m["WkT"]) + f32a(m["bk"])).astype(BF16).astype(np.float32)
    V = (feats @ f32a(m["WvT"]) + f32a(m["bv"])).astype(BF16).astype(np.float32)
    Q = (f32a(m["featsLT"]).T @ f32a(m["WqT"]) + f32a(m["bq"])).astype(BF16).astype(np.float32)

    outT = np.zeros((128, g.NLOC_PAD), np.float32)
    for b in range(g.NBLK):
        W = g.WIDTHS[b]
        base = g.BASES[b]
        pa = np.zeros((128, 128), np.float32)
        pd = np.zeros((128, g.H), np.float32)
        subs = [g.ASTART[b] + i for i in range(g.SA[b])] + [
            g.BSTART[b] + i for i in range(g.SB[b])
        ]
        qwin = Q[base : base + 128]
        for si, sub in enumerate(subs):
            half = 0 if si < g.SA[b] else 1
            idx = np.array([m["kvidx"][j % 16, sub * 8 + j // 16]
                            for j in range(128)], np.int64)
            relv = np.array([float(m["dstrel"][j, sub]) for j in range(128)])
            taboff = half * g.HALF
            kg = K[taboff + idx]
            vg = V[taboff + idx]
            oh = (relv[:, None] == np.arange(128)[None, :]).astype(np.float32)
            qe = oh @ qwin
            prod = (qe.astype(BF16).astype(np.float32) * kg).astype(BF16)
            pv = prod.reshape(128, g.H, 16).astype(np.float32)
            t1 = (pv[:, :, 0:8] + pv[:, :, 8:16]).astype(BF16).astype(np.float32)
            t2 = (t1[:, :, 0:4] + t1[:, :, 4:8]).astype(BF16).astype(np.float32)
            t3 = (t2[:, :, 0:2] + t2[:, :, 2:4]).astype(BF16).astype(np.float32)
            sc = (t3[:, :, 0] + t3[:, :, 1])
            w_ = np.exp(0.25 * sc).astype(BF16).astype(np.float32)
            wv = (vg.reshape(128, 16, 8) * w_[:, None, :]).astype(BF16).astype(
                np.float32).reshape(128, 128)
            pa += oh.T @ wv
            pd += oh.T @ w_
        cmv = m["cm_t"][:, b].astype(np.float32)
        dent = (pd + 1.0) * cmv[:, None]
        fac = 1.0 / dent
        agf = (pa.reshape(128, 16, g.H) * fac[:, None, :]).reshape(128, 128)
        po = agf @ f32a(m["WoT"]) + f32a(m["bo"])
        outT[:, base : base + W] = po.T[:, :W]
    return outT


def golden_full(g: Geom, maps):
    outs = [golden_core(g, m) for m in maps]
    return np.concatenate([o[:, : g.NLOC].T for o in outs], axis=0)


# ---------------------------------------------------------------------------
# Bass program
# ---------------------------------------------------------------------------
def build_bass(g: Geom):
    import os
    from contextlib import ExitStack

    import concourse.bacc as bacc
    import concourse.mybir as mybir
    import concourse.tile as tile
    from concourse.library_config import mlp

    f32 = mybir.dt.float32
    bf = mybir.dt.bfloat16
    i16 = mybir.dt.int16
    AL = mybir.AluOpType
    ACT = mybir.ActivationFunctionType

    nc = bacc.Bacc("TRN2", target_bir_lowering=False, num_devices=g.P,
                   dynamic_dma_scratch_size=40960)

    featsT = nc.dram_tensor("featsT", [128, g.N_TAB], bf, kind="ExternalInput")
    featsLT = nc.dram_tensor("featsLT", [128, g.QROWS], bf, kind="ExternalInput")
    wts = {
        n: nc.dram_tensor(n, [g.D, g.D], f32 if n == "WoT" else bf,
                          kind="ExternalInput")
        for n in ("WqT", "WkT", "WvT", "WoT")
    }
    bias = {
        n: nc.dram_tensor(n, [1, g.D], f32 if n == "bo" else bf,
                          kind="ExternalInput")
        for n in ("bq", "bk", "bv", "bo")
    }
    kvidx_d = nc.dram_tensor("kvidx", [128, g.TOTSUB * 8], i16,
                             kind="ExternalInput")
    dstrel_d = nc.dram_tensor("dstrel", [128, g.TOTSUB], f32,
                              kind="ExternalInput")
    cm_d = nc.dram_tensor("cm_t", [128, g.NBLK], f32, kind="ExternalInput")
    iota_d = nc.dram_tensor("iota_row", [128, 128], bf, kind="ExternalInput")
    ident_d = nc.dram_tensor("ident", [128, 128], f32, kind="ExternalInput")
    identbf_d = nc.dram_tensor("ident_bf", [128, 128], bf, kind="ExternalInput")
    ones_d = nc.dram_tensor("ones_row", [1, 128], f32, kind="ExternalInput")
    onesbf_d = nc.dram_tensor("ones_bf", [1, 128], bf, kind="ExternalInput")

    outT = nc.dram_tensor("outT", [128, g.NLOC_PAD], f32, kind="ExternalOutput")

    KV_h = [
        nc.dram_tensor(f"KV_tab{i}", [g.HALF, 2 * g.D], bf) for i in range(2)
    ]

    # 1024-column phase-1 chunks (8 matmul slots each)
    def chunks_of(total, width=1024):
        out, o = [], 0
        while o < total:
            w = min(width, total - o)
            out.append((o, w))
            o += w
        return out

    KV_CHUNKS = [(h * g.HALF + o, w) for h in (0, 1)
                 for o, w in chunks_of(g.HALF)]
    Q_CHUNKS = chunks_of(g.QROWS)

    with tile.TileContext(nc) as tc, ExitStack() as ctx:
        nc.gpsimd.load_library(mlp)

        cnt_regs = {}
        for x in set(list(g.SA) + list(g.SB)):
            n = x * 128
            r = nc.alloc_register(mybir.EngineType.Pool, f"rg{n}")
            nc.gpsimd.reg_mov(r, n)
            cnt_regs[n] = r

        const = ctx.enter_context(tc.tile_pool(name="const", bufs=1))
        w_t = {
            n: const.tile([g.D, g.D], f32 if n == "WoT" else bf, tag=n,
                          name=n + "_t")
            for n in wts
        }
        for n in wts:
            nc.sync.dma_start(w_t[n][:], wts[n][:])
        b_t = {
            n: const.tile([1, g.D], f32 if n == "bo" else bf, tag=n,
                          name=n + "_t")
            for n in bias
        }
        for n in bias:
            nc.sync.dma_start(b_t[n][:], bias[n][:])
        iota_t = const.tile([128, 128], bf, tag="iota")
        nc.sync.dma_start(iota_t[:], iota_d[:])
        id_t = const.tile([128, 128], f32, tag="ident")
        nc.sync.dma_start(id_t[:], ident_d[:])
        idbf_t = const.tile([128, 128], bf, tag="identbf")
        nc.sync.dma_start(idbf_t[:], identbf_d[:])
        ones_t = const.tile([1, 128], f32, tag="ones")
        nc.sync.dma_start(ones_t[:], ones_d[:])
        onesbf_t = const.tile([1, 128], bf, tag="onesbf")
        nc.sync.dma_start(onesbf_t[:], onesbf_d[:])
        kvidx_t = const.tile([128, g.TOTSUB * 8], i16, tag="kvidx")
        nc.sync.dma_start(kvidx_t[:], kvidx_d[:])
        dstrel_t = const.tile([128, g.TOTSUB], f32, tag="dstrel")
        nc.sync.dma_start(dstrel_t[:], dstrel_d[:])
        cm_t = const.tile([128, g.NBLK], f32, tag="cm")
        nc.sync.dma_start(cm_t[:], cm_d[:])

        q_pool = ctx.enter_context(tc.tile_pool(name="qsb", bufs=1))
        Q_sb = q_pool.tile([128, g.QROWS // 128, 128], bf, tag="Q_sb",
                           name="Q_sb")

        # ---------------- Phase 1: projections --------------------------
        with (
            tc.tile_pool(name="p1", bufs=4) as p1,
            tc.tile_pool(name="p1ps", bufs=2, space="PSUM") as p1ps,
        ):
            KV_rows = [
                t[:].rearrange("(c p) e -> p c e", p=128) for t in KV_h
            ]

            # Q first (written straight to SBUF)
            for o, w in Q_CHUNKS:
                nj = w // 128
                ftT = p1.tile([128, 1024], bf, tag="ftTq", name="ftTq")
                nc.sync.dma_start(ftT[:, :w], featsLT[:, o : o + w])
                ps = p1ps.tile([128, 8, 128], f32, tag="ps", name="psq")
                for j in range(nj):
                    if not g.ZERO_BIAS:
                        nc.tensor.matmul(ps[:, j, :], onesbf_t[:], b_t["bq"][:],
                                         start=True, stop=False)
                    nc.tensor.matmul(
                        ps[:, j, :], ftT[:, 128 * j : 128 * (j + 1)],
                        w_t["WqT"][:], start=g.ZERO_BIAS, stop=True,
                    )
                nc.scalar.activation(Q_sb[:, o // 128 : o // 128 + nj, :],
                                     ps[:, :nj, :], ACT.Copy)

            # K/V for the full table; half A first (gates A-stream gathers)
            for o, w in KV_CHUNKS:
                nj = w // 128
                half = 1 if o >= g.HALF else 0
                oh_ = o - half * g.HALF
                assert oh_ + w <= g.HALF
                ftT = p1.tile([128, 1024], bf, tag="ftT", name="ftTkv")
                nc.sync.dma_start(ftT[:, :w], featsT[:, o : o + w])
                cp = p1.tile([128, 8, 2, 128], bf, tag="cp", name="cp")
                for slot, (wn, bn) in enumerate((("WkT", "bk"), ("WvT", "bv"))):
                    ps = p1ps.tile([128, 8, 128], f32, tag="ps",
                                   name="ps" + wn)
                    for j in range(nj):
                        if not g.ZERO_BIAS:
                            nc.tensor.matmul(ps[:, j, :], onesbf_t[:],
                                             b_t[bn][:], start=True, stop=False)
                        nc.tensor.matmul(
                            ps[:, j, :], ftT[:, 128 * j : 128 * (j + 1)],
                            w_t[wn][:], start=g.ZERO_BIAS, stop=True,
                        )
                    if slot == 0:
                        nc.vector.tensor_copy(cp[:, :nj, slot, :], ps[:, :nj, :])
                    else:
                        nc.scalar.activation(cp[:, :nj, slot, :], ps[:, :nj, :],
                                             ACT.Copy)
                eng = nc.scalar if half == 0 else nc.sync
                eng.dma_start(
                    KV_rows[half][:, oh_ // 128 : oh_ // 128 + nj, :],
                    cp[:, :nj, :, :].rearrange("p c s d -> p c (s d)"),
                )

        # ---------------- Phase 2: block pairs ---------------------------
        with (
            tc.tile_pool(name="gat", bufs=2) as gat,
            tc.tile_pool(name="ohtp", bufs=4) as ohtp,
            tc.tile_pool(name="qwp", bufs=8) as qwp,
            tc.tile_pool(name="ew", bufs=2) as ew,
            tc.tile_pool(name="sm", bufs=2) as sm,
            tc.tile_pool(name="ohp", bufs=2, space="PSUM") as ohp,
            tc.tile_pool(name="qep", bufs=2, space="PSUM") as qep,
            tc.tile_pool(name="accp", bufs=2, space="PSUM") as accp,
        ):
            GMAX = max(max(la, lb) for _, _, la, _, lb in g.PAIRS)
            for blks, ga, la, gb, lb in g.PAIRS:
                # one gather per stream for the whole pair
                kvgA = gat.tile([128, GMAX, 2, 128], bf, tag="kvgA",
                                name="kvgA")
                nc.gpsimd.dma_gather(
                    kvgA[:, :la, :, :].rearrange("p c two d -> p c (two d)"),
                    KV_h[0][:], kvidx_t[:, ga * 8 : (ga + la) * 8],
                    la * 128, cnt_regs[la * 128], 2 * g.D, queue_num=0,
                )
                kvgB = gat.tile([128, GMAX, 2, 128], bf, tag="kvgB",
                                name="kvgB")
                nc.gpsimd.dma_gather(
                    kvgB[:, :lb, :, :].rearrange("p c two d -> p c (two d)"),
                    KV_h[1][:], kvidx_t[:, gb * 8 : (gb + lb) * 8],
                    lb * 128, cnt_regs[lb * 128], 2 * g.D, queue_num=0,
                )

                for b in blks:
                    S = g.S[b]
                    # (stream-half, kvg tile, kvg offset, global slot) per sub
                    subs = [
                        (0, kvgA, g.ASTART[b] - ga + i, g.ASTART[b] + i)
                        for i in range(g.SA[b])
                    ] + [
                        (1, kvgB, g.BSTART[b] - gb + i, g.BSTART[b] + i)
                        for i in range(g.SB[b])
                    ]

                    oh = ew.tile([128, g.SMAX, 128], bf, tag="oh", name="oh")
                    for si, (_, _, _, gs) in enumerate(subs):
                        nc.vector.tensor_scalar(
                            oh[:, si, :], iota_t[:],
                            dstrel_t[:, gs : gs + 1], None, AL.is_equal,
                        )

                    ohT = ew.tile([128, g.SMAX, 128], bf, tag="ohT", name="ohT")
                    qe = ew.tile([128, g.SMAX, 128], bf, tag="qe", name="qe")
                    c0 = 0
                    ci = 0
                    while c0 < S:
                        cw = min(g.CH, S - c0)
                        ohT_ps = ohp.tile([128, g.CH, 128], bf, tag="ohTps",
                                          name="ohTps")
                        for s in range(cw):
                            nc.tensor.transpose(ohT_ps[:, s, :],
                                                oh[:, c0 + s, :], idbf_t[:])
                        if ci % 3 == 0:
                            nc.scalar.activation(ohT[:, c0 : c0 + cw, :],
                                                 ohT_ps[:, :cw, :], ACT.Copy)
                        else:
                            nc.gpsimd.tensor_copy(ohT[:, c0 : c0 + cw, :],
                                                  ohT_ps[:, :cw, :])
                        qe_ps = qep.tile([128, g.CH, 128], f32, tag="qeps",
                                         name="qeps")
                        for s in range(cw):
                            nc.tensor.matmul(qe_ps[:, s, :], ohT[:, c0 + s, :],
                                             Q_sb[:, b, :], start=True,
                                             stop=True)
                        nc.scalar.activation(qe[:, c0 : c0 + cw, :],
                                             qe_ps[:, :cw, :], ACT.Copy)
                        c0 += cw
                        ci += 1

                    # prod = qe * k (2x TT), per stream run
                    prod = ew.tile([128, g.SMAX, 128], bf, tag="prod",
                                   name="prod")
                    for half, Sx, o in ((0, g.SA[b], 0), (1, g.SB[b], g.SA[b])):
                        kvt, ko = (kvgA, g.ASTART[b] - ga) if half == 0 else (
                            kvgB, g.BSTART[b] - gb)
                        nc.vector.tensor_tensor(
                            prod[:, o : o + Sx, :], qe[:, o : o + Sx, :],
                            kvt[:, ko : ko + Sx, 0, :], AL.mult,
                        )

                    pv = prod[:, :S, :].rearrange("p s (h d) -> p s h d", d=16)
                    t1 = sm.tile([128, g.SMAX, 8, 8], bf, tag="t1", name="t1")
                    nc.vector.tensor_tensor(
                        t1[:, :S], pv[:, :, :, 0:8], pv[:, :, :, 8:16], AL.add)
                    t2 = sm.tile([128, g.SMAX, 8, 4], bf, tag="t2", name="t2")
                    nc.vector.tensor_tensor(
                        t2[:, :S], t1[:, :S, :, 0:4], t1[:, :S, :, 4:8], AL.add)
                    t3 = sm.tile([128, g.SMAX, 8, 2], bf, tag="t3", name="t3")
                    nc.vector.tensor_tensor(
                        t3[:, :S], t2[:, :S, :, 0:2], t2[:, :S, :, 2:4], AL.add)
                    sc = sm.tile([128, g.SMAX, 8], f32, tag="sc", name="sc")
                    nc.vector.tensor_tensor(
                        sc[:, :S], t3[:, :S, :, 0], t3[:, :S, :, 1], AL.add)

                    # wv cols 0:128 = v * w (v hd-major); cols 128:136 = w
                    wv = ew.tile([128, g.SMAX, 136], bf, tag="wv", name="wv")
                    wexp = wv[:, :, 128:136]
                    nc.scalar.activation(wexp[:, :S], sc[:, :S], ACT.Exp,
                                         scale=0.25)
                    for half, Sx, o in ((0, g.SA[b], 0), (1, g.SB[b], g.SA[b])):
                        kvt, ko = (kvgA, g.ASTART[b] - ga) if half == 0 else (
                            kvgB, g.BSTART[b] - gb)
                        nc.vector.tensor_tensor(
                            wv[:, o : o + Sx, 0:128].rearrange(
                                "p s (d h) -> p s d h", h=8),
                            kvt[:, ko : ko + Sx, 1, :].rearrange(
                                "p s (d h) -> p s d h", h=8),
                            wexp[:, o : o + Sx, :]
                            .rearrange("p s (o h) -> p s o h", o=1)
                            .broadcast_to([128, Sx, 16, 8]),
                            AL.mult,
                        )

                    # aggregate: one matmul per subtile -> [pa | pd]
                    accfin = accp.tile([128, 392], f32, tag="accfin",
                                       name="accfin")
                    pa = accfin[:, 0:128]
                    pd = accfin[:, 128:136]
                    for s in range(S):
                        nc.tensor.matmul(accfin[:, 0:136], oh[:, s, :],
                                         wv[:, s, :],
                                         start=(s == 0), stop=(s == S - 1))

                    # finalize
                    dent = sm.tile([128, 8], f32, tag="dent", name="dent")
                    nc.vector.scalar_tensor_tensor(
                        dent[:], pd, 1.0,
                        cm_t[:, b : b + 1].broadcast_to([128, 8]),
                        AL.add, AL.mult,
                    )
                    fac = sm.tile([128, 8], f32, tag="fac", name="fac")
                    nc.vector.reciprocal(fac[:], dent[:])
                    agf = sm.tile([128, 128], f32, tag="agf", name="agf")
                    nc.vector.scalar_tensor_tensor(
                        agf[:].rearrange("p (d h) -> p d h", h=8),
                        pa.rearrange("p (d h) -> p d h", h=8),
                        1.0,
                        fac[:].rearrange("p (o h) -> p o h", o=1)
                        .broadcast_to([128, 16, 8]),
                        AL.mult, AL.mult,
                    )
                    agfT_ps = accfin[:, 136:264]
                    nc.tensor.transpose(agfT_ps, agf[:], id_t[:])
                    agfT = sm.tile([128, 128], f32, tag="agfTs", name="agfTs")
                    nc.scalar.activation(agfT[:], agfT_ps, ACT.Copy)
                    po = accfin[:, 264:392]
                    if not g.ZERO_BIAS:
                        nc.tensor.matmul(po, b_t["bo"][:], ones_t[:],
                                         start=True, stop=False)
                    nc.tensor.matmul(po, w_t["WoT"][:], agfT[:],
                                     start=g.ZERO_BIAS, stop=True)
                    oc = sm.tile([128, 128], f32, tag="oc", name="oc")
                    nc.scalar.activation(oc[:], po, ACT.Copy)
                    nc.sync.dma_start(outT[:, b * 128 : (b + 1) * 128], oc[:])

    nc.compile()
    return nc


# ---------------------------------------------------------------------------
# Entry point
# ---------------------------------------------------------------------------
N_NODES = 50000
N_CORES = 8

_CACHE = {}


def kernel(**inputs):
    from concourse.bass_utils import run_bass_kernel_spmd

    feats = np.asarray(inputs["feats"], np.float32)
    edge_index = np.asarray(inputs["edge_index"], np.int64)
    src = edge_index[:, 0]
    dst = edge_index[:, 1]

    zb = all(
        not np.any(np.asarray(inputs[k])) for k in ("bq", "bk", "bv")
    )
    SA, SB = compute_schedule(N_NODES, N_CORES, src, dst)
    g = Geom(N_NODES, N_CORES, SA, SB, zero_bias=zb)

    maps = host_prep(
        g, feats, edge_index,
        np.asarray(inputs["Wq"], np.float32), np.asarray(inputs["bq"], np.float32),
        np.asarray(inputs["Wk"], np.float32), np.asarray(inputs["bk"], np.float32),
        np.asarray(inputs["Wv"], np.float32), np.asarray(inputs["bv"], np.float32),
        np.asarray(inputs["Wo"], np.float32), np.asarray(inputs["bo"], np.float32),
    )

    key = (SA, SB, zb)
    if key not in _CACHE:
        _CACHE[key] = build_bass(g)
    nc = _CACHE[key]

    res = run_bass_kernel_spmd(nc, maps, list(range(N_CORES)))
    out = np.empty((N_NODES, g.D), np.float32)
    for c in range(N_CORES):
        out[c * g.NLOC : (c + 1) * g.NLOC] = res.results[c]["outT"][:, : g.NLOC].T
    return out


# revision 4
# speedup vs baseline: 1.1064x; 1.0048x over previous
"""Trainium2 Bass kernel v3: multi-head dot-product GNN message passing.

Self-contained: accepts FULL inputs, shards destinations across 8 NeuronCores,
returns the FULL [50000, 128] output.

Design (block-resident partials, no DRAM accumulator):
- Destinations sharded across cores (NLOC each); each core's edges grouped by
  128-aligned dst BLOCK, split into two streams by source half (gather idx is
  int16, table has 50176 rows). Per (block, stream) the edge count is padded to
  a multiple of 128; the static per-block schedule (SA[b], SB[b]) is the max
  over cores so one program serves all cores (SPMD). Blocks are processed in
  PAIRS sharing one gather per stream (slot layout: A[b0] A[b1] B[b0] B[b1]).
- Per block: build per-subtile one-hot oh[e,d] (tensor_scalar 4x), PE-transpose
  to ohT[d,e], expand per-edge Q via Qe = ohT^T @ Qblk on the PE (Q lives in
  SBUF, node-major, 128-aligned blocks); scores via 2x TT multiply +
  binary-tree head reduce; exp on ACT; V is stored hd-major so the exp-weight
  broadcast keeps innermost packing (2x TT); aggregate [pa|pd] in PSUM via one
  one-hot matmul per subtile (partials are FINAL: every dst lives in exactly
  one block); finalize in place and write the output block. No scatter-add, no
  accumulator zero/readback, no per-edge Q gather.

Per-edge math (identical to reference's clamped scatter-softmax):
  attn[e,h] = exp(s)/(1 + sum_seg exp(s'))      [max-shift cancels exactly]
  out[n]    = (sum exp(s) * v[src]) / (1+den) / max(cnt,1) @ Wo.T + bo
"""

import numpy as np
import ml_dtypes

BF16 = ml_dtypes.bfloat16
SENT = 30000.0  # one-hot sentinel (never matches iota 0..127)

# V/Wo head-dim-major permutation: col j=(hd*8+h) <- col h*16+hd
PERM = np.array([(j % 8) * 16 + j // 8 for j in range(128)], np.int64)


# ---------------------------------------------------------------------------
# Geometry + static schedule
# ---------------------------------------------------------------------------
class Geom:
    def __init__(self, n_nodes, n_cores, bases, sched_a, sched_b, d=128, h=8,
                 zero_bias=False):
        self.ZERO_BIAS = zero_bias
        self.N = n_nodes
        self.P = n_cores
        self.D = d
        self.H = h
        self.HD = d // h
        assert n_nodes % n_cores == 0
        self.NLOC = n_nodes // n_cores
        self.NLOC_PAD = ((self.NLOC + 127) // 128) * 128
        self.N_TAB = ((n_nodes + 1023) // 1024) * 1024
        self.HALF = self.N_TAB // 2
        assert self.HALF - 1 <= 32767
        self.QROWS = ((self.NLOC_PAD + 511) // 512) * 512
        self.BASES = tuple(int(x) for x in bases)   # shared block bases
        self.NBLK = len(self.BASES)
        self.WIDTHS = tuple(
            (self.BASES[i + 1] if i + 1 < self.NBLK else self.NLOC)
            - self.BASES[i]
            for i in range(self.NBLK)
        )
        assert all(0 < w <= 128 for w in self.WIDTHS)
        self.SA = tuple(int(x) for x in sched_a)
        self.SB = tuple(int(x) for x in sched_b)
        assert len(self.SA) == self.NBLK and len(self.SB) == self.NBLK
        assert max(self.SA) <= 8 and max(self.SB) <= 8  # gather <= 1024 idx
        self.S = tuple(a + b for a, b in zip(self.SA, self.SB))
        self.SMAX = max(self.S)
        # pair-grouped slot layout: for pair (b0, b1): A[b0] A[b1] B[b0] B[b1]
        self.PAIRS = []
        astart = [0] * self.NBLK
        bstart = [0] * self.NBLK
        off = 0
        b = 0
        while b < self.NBLK:
            blks = [b] if b + 1 >= self.NBLK else [b, b + 1]
            ga_start = off
            for bb in blks:
                astart[bb] = off
                off += self.SA[bb]
            gb_start = off
            for bb in blks:
                bstart[bb] = off
                off += self.SB[bb]
            self.PAIRS.append(
                (blks, ga_start, gb_start - ga_start, gb_start, off - gb_start)
            )
            b += 2
        self.ASTART = tuple(astart)
        self.BSTART = tuple(bstart)
        self.TOTSUB = off
        self.CH = 4  # subtile chunk size (PSUM staging granularity)


def compute_schedule(n_nodes, n_cores, src, dst, cap=1024):
    """Per-core node permutation (snake order by degree) + shared
    variable-width block cuts: every (core, block, stream) count <= cap
    so each stream fits one dma_gather call (ucode limit 1024)."""
    NLOC = n_nodes // n_cores
    N_TAB = ((n_nodes + 1023) // 1024) * 1024
    HALF = N_TAB // 2
    degA = np.zeros((n_cores, NLOC), np.int64)
    degB = np.zeros((n_cores, NLOC), np.int64)
    for c in range(n_cores):
        lo = c * NLOC
        m = (dst >= lo) & (dst < lo + NLOC)
        sloc, d = src[m], dst[m] - lo
        hB = sloc >= HALF
        np.add.at(degA[c], d[~hB], 1)
        np.add.at(degB[c], d[hB], 1)
    # snake: sort by total degree, deal alternately front/back so cumulative
    # sums grow smoothly and cuts land on full 128-wide blocks
    perms = np.zeros((n_cores, NLOC), np.int64)
    for c in range(n_cores):
        order = np.argsort(degA[c] + degB[c], kind="stable")[::-1]
        snake = np.empty(NLOC, np.int64)
        snake[0::2] = order[: (NLOC + 1) // 2]
        snake[1::2] = order[(NLOC + 1) // 2 :][::-1]
        perms[c] = snake            # position i holds original-local node id
    pdA = np.take_along_axis(degA, perms, axis=1)
    pdB = np.take_along_axis(degB, perms, axis=1)
    bases, base = [0], 0
    accA = np.zeros(n_cores, np.int64)
    accB = np.zeros(n_cores, np.int64)
    nA_blocks, nB_blocks = [], []
    for n in range(NLOC):
        w = n - base
        if (w >= 128 or (accA + pdA[:, n]).max() > cap
                or (accB + pdB[:, n]).max() > cap):
            nA_blocks.append(accA.copy())
            nB_blocks.append(accB.copy())
            bases.append(n)
            base = n
            accA[:] = 0
            accB[:] = 0
        accA += pdA[:, n]
        accB += pdB[:, n]
    nA_blocks.append(accA.copy())
    nB_blocks.append(accB.copy())
    SA = [max(1, int(np.ceil(a.max() / 128))) for a in nA_blocks]
    SB = [max(1, int(np.ceil(b.max() / 128))) for b in nB_blocks]
    return tuple(bases), tuple(SA), tuple(SB), perms


# ---------------------------------------------------------------------------
# Host-side packing
# ---------------------------------------------------------------------------
def pack_core(g: Geom, src, dst, core, perm):
    """Per-core kvidx [128, TOTSUB*8] i16 and dstrel [128, TOTSUB] f32."""
    lo = core * g.NLOC
    m = (dst >= lo) & (dst < lo + g.NLOC)
    inv = np.empty(g.NLOC, np.int64)
    inv[perm] = np.arange(g.NLOC)
    s, d = src[m].astype(np.int64), inv[(dst[m] - lo).astype(np.int64)]
    blk = np.searchsorted(np.array(g.BASES), d, side="right") - 1
    rel = d - np.array(g.BASES)[blk]

    cnt = np.bincount(d, minlength=g.NLOC).astype(np.float32)
    cm = np.ones((128, g.NBLK), np.float32)
    for b in range(g.NBLK):
        w = g.WIDTHS[b]
        cm[:w, b] = np.maximum(cnt[g.BASES[b] : g.BASES[b] + w], 1.0)

    kvidx = np.zeros((g.TOTSUB * 128,), np.int16)
    dstrel = np.full((g.TOTSUB * 128,), SENT, np.float32)

    for b in range(g.NBLK):
        mb = blk == b
        sb_, rb_ = s[mb], rel[mb]
        hB = sb_ >= g.HALF
        for half, (ss, rr) in enumerate(
            ((sb_[~hB], rb_[~hB]), (sb_[hB] - g.HALF, rb_[hB]))
        ):
            off = g.ASTART[b] if half == 0 else g.BSTART[b]
            nslot = (g.SA[b] if half == 0 else g.SB[b]) * 128
            n = len(ss)
            assert n <= nslot, (core, b, half, n, nslot)
            base = off * 128
            kvidx[base : base + n] = ss.astype(np.int16)
            dstrel[base : base + n] = rr.astype(np.float32)

    kvw = np.zeros((128, g.TOTSUB * 8), np.int16)
    kvw[0:16] = kvidx.reshape(-1, 16).T
    for k in range(1, 8):
        kvw[16 * k : 16 * (k + 1)] = kvw[0:16]
    drl = dstrel.reshape(g.TOTSUB, 128).T.astype(np.float32).copy()
    return dict(kvidx=kvw, dstrel=drl,
                dstrel_row=dstrel.astype(BF16).reshape(1, -1), cm_t=cm)


def host_prep(g: Geom, perms, feats, edge_index, Wq, bq, Wk, bk, Wv, bv, Wo, bo):
    src = np.asarray(edge_index[:, 0], np.int64)
    dst = np.asarray(edge_index[:, 1], np.int64)
    feats = np.asarray(feats, np.float32)

    feats_pad = np.zeros((g.N_TAB, g.D), np.float32)
    feats_pad[: g.N] = feats
    featsT = np.ascontiguousarray(feats_pad.T)

    iota_row = np.tile(np.arange(128, dtype=np.float32)[None, :], (128, 1))

    WvTp = np.ascontiguousarray(Wv.T[:, PERM])   # V output cols hd-major
    WoTp = np.ascontiguousarray(Wo.T[PERM, :])   # Wo input rows hd-major

    common = dict(
        featsT=featsT.astype(BF16),
        WqT=np.ascontiguousarray(Wq.T.astype(BF16)),
        WkT=np.ascontiguousarray(Wk.T.astype(BF16)),
        WvT=WvTp.astype(BF16),
        WoT=WoTp.astype(np.float32),
        bq=bq.astype(BF16).reshape(1, g.D),
        bk=bk.astype(BF16).reshape(1, g.D),
        bv=bv[PERM].astype(BF16).reshape(1, g.D),
        bo=bo.astype(np.float32).reshape(1, g.D),
        iota_row=iota_row.astype(BF16),
        ident=np.eye(128, dtype=np.float32),
        ident_bf=np.eye(128, dtype=np.float32).astype(BF16),
        ones_row=np.ones((1, 128), np.float32),
        ones_bf=np.ones((1, 128), np.float32).astype(BF16),
    )

    maps = []
    for c in range(g.P):
        featsL = np.zeros((g.QROWS, g.D), np.float32)
        featsL[: g.NLOC] = feats[c * g.NLOC : (c + 1) * g.NLOC][perms[c]]
        mc = dict(common)
        mc["featsLT"] = np.ascontiguousarray(featsL.T.astype(BF16))
        mc.update(pack_core(g, src, dst, c, perms[c]))
        maps.append(mc)
    return maps


# ---------------------------------------------------------------------------
# Numpy golden model of the DEVICE algorithm
# ---------------------------------------------------------------------------
def golden_core(g: Geom, m):
    f32a = lambda x: np.asarray(x, np.float32)
    feats = f32a(m["featsT"]).T
    K = (feats @ f32a(m["WkT"]) + f32a(m["bk"])).astype(BF16).astype(np.float32)
    V = (feats @ f32a(m["WvT"]) + f32a(m["bv"])).astype(BF16).astype(np.float32)
    Q = (f32a(m["featsLT"]).T @ f32a(m["WqT"]) + f32a(m["bq"])).astype(BF16).astype(np.float32)

    outT = np.zeros((128, g.NLOC_PAD), np.float32)
    for b in range(g.NBLK):
        W = g.WIDTHS[b]
        base = g.BASES[b]
        pa = np.zeros((128, 128), np.float32)
        pd = np.zeros((128, g.H), np.float32)
        subs = [g.ASTART[b] + i for i in range(g.SA[b])] + [
            g.BSTART[b] + i for i in range(g.SB[b])
        ]
        qwin = Q[base : base + 128]
        for si, sub in enumerate(subs):
            half = 0 if si < g.SA[b] else 1
            idx = np.array([m["kvidx"][j % 16, sub * 8 + j // 16]
                            for j in range(128)], np.int64)
            relv = np.array([float(m["dstrel"][j, sub]) for j in range(128)])
            taboff = half * g.HALF
            kg = K[taboff + idx]
            vg = V[taboff + idx]
            oh = (relv[:, None] == np.arange(128)[None, :]).astype(np.float32)
            qe = oh @ qwin
            prod = (qe.astype(BF16).astype(np.float32) * kg).astype(BF16)
            pv = prod.reshape(128, g.H, 16).astype(np.float32)
            t1 = (pv[:, :, 0:8] + pv[:, :, 8:16]).astype(BF16).astype(np.float32)
            t2 = (t1[:, :, 0:4] + t1[:, :, 4:8]).astype(BF16).astype(np.float32)
            t3 = (t2[:, :, 0:2] + t2[:, :, 2:4]).astype(BF16).astype(np.float32)
            sc = (t3[:, :, 0] + t3[:, :, 1])
            w_ = np.exp(0.25 * sc).astype(BF16).astype(np.float32)
            wv = (vg.reshape(128, 16, 8) * w_[:, None, :]).astype(BF16).astype(
                np.float32).reshape(128, 128)
            pa += oh.T @ wv
            pd += oh.T @ w_
        cmv = m["cm_t"][:, b].astype(np.float32)
        dent = (pd + 1.0) * cmv[:, None]
        fac = 1.0 / dent
        agf = (pa.reshape(128, 16, g.H) * fac[:, None, :]).reshape(128, 128)
        po = agf @ f32a(m["WoT"]) + f32a(m["bo"])
        outT[:, base : base + W] = po.T[:, :W]
    return outT


def golden_full(g: Geom, maps):
    outs = [golden_core(g, m) for m in maps]
    return np.concatenate([o[:, : g.NLOC].T for o in outs], axis=0)


# ---------------------------------------------------------------------------
# Bass program
# ---------------------------------------------------------------------------
def build_bass(g: Geom):
    import os
    from contextlib import ExitStack

    import concourse.bacc as bacc
    import concourse.mybir as mybir
    import concourse.tile as tile
    from concourse.library_config import mlp

    f32 = mybir.dt.float32
    bf = mybir.dt.bfloat16
    i16 = mybir.dt.int16
    AL = mybir.AluOpType
    ACT = mybir.ActivationFunctionType

    nc = bacc.Bacc("TRN2", target_bir_lowering=False, num_devices=g.P,
                   dynamic_dma_scratch_size=40960)

    featsT = nc.dram_tensor("featsT", [128, g.N_TAB], bf, kind="ExternalInput")
    featsLT = nc.dram_tensor("featsLT", [128, g.QROWS], bf, kind="ExternalInput")
    wts = {
        n: nc.dram_tensor(n, [g.D, g.D], f32 if n == "WoT" else bf,
                          kind="ExternalInput")
        for n in ("WqT", "WkT", "WvT", "WoT")
    }
    bias = {
        n: nc.dram_tensor(n, [1, g.D], f32 if n == "bo" else bf,
                          kind="ExternalInput")
        for n in ("bq", "bk", "bv", "bo")
    }
    kvidx_d = nc.dram_tensor("kvidx", [128, g.TOTSUB * 8], i16,
                             kind="ExternalInput")
    dstrel_d = nc.dram_tensor("dstrel", [128, g.TOTSUB], f32,
                              kind="ExternalInput")
    cm_d = nc.dram_tensor("cm_t", [128, g.NBLK], f32, kind="ExternalInput")
    iota_d = nc.dram_tensor("iota_row", [128, 128], bf, kind="ExternalInput")
    ident_d = nc.dram_tensor("ident", [128, 128], f32, kind="ExternalInput")
    identbf_d = nc.dram_tensor("ident_bf", [128, 128], bf, kind="ExternalInput")
    ones_d = nc.dram_tensor("ones_row", [1, 128], f32, kind="ExternalInput")
    onesbf_d = nc.dram_tensor("ones_bf", [1, 128], bf, kind="ExternalInput")

    outT = nc.dram_tensor("outT", [128, g.NLOC_PAD], f32, kind="ExternalOutput")

    KV_h = [
        nc.dram_tensor(f"KV_tab{i}", [g.HALF, 2 * g.D], bf) for i in range(2)
    ]

    # 1024-column phase-1 chunks (8 matmul slots each)
    def chunks_of(total, width=1024):
        out, o = [], 0
        while o < total:
            w = min(width, total - o)
            out.append((o, w))
            o += w
        return out

    KV_CHUNKS = [(h * g.HALF + o, w) for h in (0, 1)
                 for o, w in chunks_of(g.HALF)]
    Q_CHUNKS = chunks_of(g.QROWS)

    with tile.TileContext(nc) as tc, ExitStack() as ctx:
        nc.gpsimd.load_library(mlp)

        cnt_regs = {}
        for x in set(list(g.SA) + list(g.SB)):
            n = x * 128
            r = nc.alloc_register(mybir.EngineType.Pool, f"rg{n}")
            nc.gpsimd.reg_mov(r, n)
            cnt_regs[n] = r

        const = ctx.enter_context(tc.tile_pool(name="const", bufs=1))
        w_t = {
            n: const.tile([g.D, g.D], f32 if n == "WoT" else bf, tag=n,
                          name=n + "_t")
            for n in wts
        }
        for n in wts:
            nc.sync.dma_start(w_t[n][:], wts[n][:])
        b_t = {
            n: const.tile([1, g.D], f32 if n == "bo" else bf, tag=n,
                          name=n + "_t")
            for n in bias
        }
        for n in bias:
            nc.sync.dma_start(b_t[n][:], bias[n][:])
        iota_t = const.tile([128, 128], bf, tag="iota")
        nc.sync.dma_start(iota_t[:], iota_d[:])
        id_t = const.tile([128, 128], f32, tag="ident")
        nc.sync.dma_start(id_t[:], ident_d[:])
        idbf_t = const.tile([128, 128], bf, tag="identbf")
        nc.sync.dma_start(idbf_t[:], identbf_d[:])
        ones_t = const.tile([1, 128], f32, tag="ones")
        nc.sync.dma_start(ones_t[:], ones_d[:])
        onesbf_t = const.tile([1, 128], bf, tag="onesbf")
        nc.sync.dma_start(onesbf_t[:], onesbf_d[:])
        kvidx_t = const.tile([128, g.TOTSUB * 8], i16, tag="kvidx")
        nc.sync.dma_start(kvidx_t[:], kvidx_d[:])
        dstrel_t = const.tile([128, g.TOTSUB], f32, tag="dstrel")
        nc.sync.dma_start(dstrel_t[:], dstrel_d[:])
        cm_t = const.tile([128, g.NBLK], f32, tag="cm")
        nc.sync.dma_start(cm_t[:], cm_d[:])

        q_pool = ctx.enter_context(tc.tile_pool(name="qsb", bufs=1))
        Q_sb = q_pool.tile([128, g.QROWS // 128, 128], bf, tag="Q_sb",
                           name="Q_sb")

        # ---------------- Phase 1: projections --------------------------
        with (
            tc.tile_pool(name="p1", bufs=4) as p1,
            tc.tile_pool(name="p1ps", bufs=2, space="PSUM") as p1ps,
        ):
            KV_rows = [
                t[:].rearrange("(c p) e -> p c e", p=128) for t in KV_h
            ]

            # Q first (written straight to SBUF)
            for o, w in Q_CHUNKS:
                nj = w // 128
                ftT = p1.tile([128, 1024], bf, tag="ftTq", name="ftTq")
                nc.sync.dma_start(ftT[:, :w], featsLT[:, o : o + w])
                ps = p1ps.tile([128, 8, 128], f32, tag="ps", name="psq")
                for j in range(nj):
                    if not g.ZERO_BIAS:
                        nc.tensor.matmul(ps[:, j, :], onesbf_t[:], b_t["bq"][:],
                                         start=True, stop=False)
                    nc.tensor.matmul(
                        ps[:, j, :], ftT[:, 128 * j : 128 * (j + 1)],
                        w_t["WqT"][:], start=g.ZERO_BIAS, stop=True,
                    )
                nc.scalar.activation(Q_sb[:, o // 128 : o // 128 + nj, :],
                                     ps[:, :nj, :], ACT.Copy)

            # K/V for the full table; half A first (gates A-stream gathers)
            for o, w in KV_CHUNKS:
                nj = w // 128
                half = 1 if o >= g.HALF else 0
                oh_ = o - half * g.HALF
                assert oh_ + w <= g.HALF
                ftT = p1.tile([128, 1024], bf, tag="ftT", name="ftTkv")
                nc.sync.dma_start(ftT[:, :w], featsT[:, o : o + w])
                cp = p1.tile([128, 8, 2, 128], bf, tag="cp", name="cp")
                for slot, (wn, bn) in enumerate((("WkT", "bk"), ("WvT", "bv"))):
                    ps = p1ps.tile([128, 8, 128], f32, tag="ps",
                                   name="ps" + wn)
                    for j in range(nj):
                        if not g.ZERO_BIAS:
                            nc.tensor.matmul(ps[:, j, :], onesbf_t[:],
                                             b_t[bn][:], start=True, stop=False)
                        nc.tensor.matmul(
                            ps[:, j, :], ftT[:, 128 * j : 128 * (j + 1)],
                            w_t[wn][:], start=g.ZERO_BIAS, stop=True,
                        )
                    if slot == 0:
                        nc.vector.tensor_copy(cp[:, :nj, slot, :], ps[:, :nj, :])
                    else:
                        nc.scalar.activation(cp[:, :nj, slot, :], ps[:, :nj, :],
                                             ACT.Copy)
                eng = nc.scalar if half == 0 else nc.sync
                eng.dma_start(
                    KV_rows[half][:, oh_ // 128 : oh_ // 128 + nj, :],
                    cp[:, :nj, :, :].rearrange("p c s d -> p c (s d)"),
                )

        # ---------------- Phase 2: block pairs ---------------------------
        with (
            tc.tile_pool(name="gat", bufs=2) as gat,
            tc.tile_pool(name="ohtp", bufs=4) as ohtp,
            tc.tile_pool(name="qwp", bufs=8) as qwp,
            tc.tile_pool(name="ew", bufs=2) as ew,
            tc.tile_pool(name="sm", bufs=2) as sm,
            tc.tile_pool(name="ohp", bufs=2, space="PSUM") as ohp,
            tc.tile_pool(name="qep", bufs=2, space="PSUM") as qep,
            tc.tile_pool(name="accp", bufs=2, space="PSUM") as accp,
        ):
            GMAX = max(max(la, lb) for _, _, la, _, lb in g.PAIRS)
            for blks, ga, la, gb, lb in g.PAIRS:
                # one gather per stream for the whole pair
                kvgA = gat.tile([128, GMAX, 2, 128], bf, tag="kvgA",
                                name="kvgA")
                nc.gpsimd.dma_gather(
                    kvgA[:, :la, :, :].rearrange("p c two d -> p c (two d)"),
                    KV_h[0][:], kvidx_t[:, ga * 8 : (ga + la) * 8],
                    la * 128, cnt_regs[la * 128], 2 * g.D, queue_num=0,
                )
                kvgB = gat.tile([128, GMAX, 2, 128], bf, tag="kvgB",
                                name="kvgB")
                nc.gpsimd.dma_gather(
                    kvgB[:, :lb, :, :].rearrange("p c two d -> p c (two d)"),
                    KV_h[1][:], kvidx_t[:, gb * 8 : (gb + lb) * 8],
                    lb * 128, cnt_regs[lb * 128], 2 * g.D, queue_num=0,
                )

                for b in blks:
                    S = g.S[b]
                    # (stream-half, kvg tile, kvg offset, global slot) per sub
                    subs = [
                        (0, kvgA, g.ASTART[b] - ga + i, g.ASTART[b] + i)
                        for i in range(g.SA[b])
                    ] + [
                        (1, kvgB, g.BSTART[b] - gb + i, g.BSTART[b] + i)
                        for i in range(g.SB[b])
                    ]

                    oh = ew.tile([128, g.SMAX, 128], bf, tag="oh", name="oh")
                    for si, (_, _, _, gs) in enumerate(subs):
                        nc.vector.tensor_scalar(
                            oh[:, si, :], iota_t[:],
                            dstrel_t[:, gs : gs + 1], None, AL.is_equal,
                        )

                    ohT = ew.tile([128, g.SMAX, 128], bf, tag="ohT", name="ohT")
                    qe = ew.tile([128, g.SMAX, 128], bf, tag="qe", name="qe")
                    c0 = 0
                    ci = 0
                    while c0 < S:
                        cw = min(g.CH, S - c0)
                        ohT_ps = ohp.tile([128, g.CH, 128], bf, tag="ohTps",
                                          name="ohTps")
                        for s in range(cw):
                            nc.tensor.transpose(ohT_ps[:, s, :],
                                                oh[:, c0 + s, :], idbf_t[:])
                        if ci % 3 == 0:
                            nc.scalar.activation(ohT[:, c0 : c0 + cw, :],
                                                 ohT_ps[:, :cw, :], ACT.Copy)
                        else:
                            nc.gpsimd.tensor_copy(ohT[:, c0 : c0 + cw, :],
                                                  ohT_ps[:, :cw, :])
                        qe_ps = qep.tile([128, g.CH, 128], f32, tag="qeps",
                                         name="qeps")
                        for s in range(cw):
                            nc.tensor.matmul(qe_ps[:, s, :], ohT[:, c0 + s, :],
                                             Q_sb[:, b, :], start=True,
                                             stop=True)
                        nc.scalar.activation(qe[:, c0 : c0 + cw, :],
                                             qe_ps[:, :cw, :], ACT.Copy)
                        c0 += cw
                        ci += 1

                    # prod = qe * k (2x TT), per stream run
                    prod = ew.tile([128, g.SMAX, 128], bf, tag="prod",
                                   name="prod")
                    for half, Sx, o in ((0, g.SA[b], 0), (1, g.SB[b], g.SA[b])):
                        kvt, ko = (kvgA, g.ASTART[b] - ga) if half == 0 else (
                            kvgB, g.BSTART[b] - gb)
                        nc.vector.tensor_tensor(
                            prod[:, o : o + Sx, :], qe[:, o : o + Sx, :],
                            kvt[:, ko : ko + Sx, 0, :], AL.mult,
                        )

                    pv = prod[:, :S, :].rearrange("p s (h d) -> p s h d", d=16)
                    t1 = sm.tile([128, g.SMAX, 8, 8], bf, tag="t1", name="t1")
                    nc.vector.tensor_tensor(
                        t1[:, :S], pv[:, :, :, 0:8], pv[:, :, :, 8:16], AL.add)
                    t2 = sm.tile([128, g.SMAX, 8, 4], bf, tag="t2", name="t2")
                    nc.vector.tensor_tensor(
                        t2[:, :S], t1[:, :S, :, 0:4], t1[:, :S, :, 4:8], AL.add)
                    t3 = sm.tile([128, g.SMAX, 8, 2], bf, tag="t3", name="t3")
                    nc.vector.tensor_tensor(
                        t3[:, :S], t2[:, :S, :, 0:2], t2[:, :S, :, 2:4], AL.add)
                    sc = sm.tile([128, g.SMAX, 8], f32, tag="sc", name="sc")
                    nc.vector.tensor_tensor(
                        sc[:, :S], t3[:, :S, :, 0], t3[:, :S, :, 1], AL.add)

                    # wv cols 0:128 = v * w (v hd-major); cols 128:136 = w
                    wv = ew.tile([128, g.SMAX, 136], bf, tag="wv", name="wv")
                    wexp = wv[:, :, 128:136]
                    nc.scalar.activation(wexp[:, :S], sc[:, :S], ACT.Exp,
                                         scale=0.25)
                    for half, Sx, o in ((0, g.SA[b], 0), (1, g.SB[b], g.SA[b])):
                        kvt, ko = (kvgA, g.ASTART[b] - ga) if half == 0 else (
                            kvgB, g.BSTART[b] - gb)
                        nc.vector.tensor_tensor(
                            wv[:, o : o + Sx, 0:128].rearrange(
                                "p s (d h) -> p s d h", h=8),
                            kvt[:, ko : ko + Sx, 1, :].rearrange(
                                "p s (d h) -> p s d h", h=8),
                            wexp[:, o : o + Sx, :]
                            .rearrange("p s (o h) -> p s o h", o=1)
                            .broadcast_to([128, Sx, 16, 8]),
                            AL.mult,
                        )

                    # aggregate: one matmul per subtile -> [pa | pd]
                    accfin = accp.tile([128, 392], f32, tag="accfin",
                                       name="accfin")
                    pa = accfin[:, 0:128]
                    pd = accfin[:, 128:136]
                    for s in range(S):
                        nc.tensor.matmul(accfin[:, 0:136], oh[:, s, :],
                                         wv[:, s, :],
                                         start=(s == 0), stop=(s == S - 1))

                    # finalize
                    dent = sm.tile([128, 8], f32, tag="dent", name="dent")
                    nc.vector.scalar_tensor_tensor(
                        dent[:], pd, 1.0,
                        cm_t[:, b : b + 1].broadcast_to([128, 8]),
                        AL.add, AL.mult,
                    )
                    fac = sm.tile([128, 8], f32, tag="fac", name="fac")
                    nc.vector.reciprocal(fac[:], dent[:])
                    agf = sm.tile([128, 128], f32, tag="agf", name="agf")
                    nc.vector.scalar_tensor_tensor(
                        agf[:].rearrange("p (d h) -> p d h", h=8),
                        pa.rearrange("p (d h) -> p d h", h=8),
                        1.0,
                        fac[:].rearrange("p (o h) -> p o h", o=1)
                        .broadcast_to([128, 16, 8]),
                        AL.mult, AL.mult,
                    )
                    agfT_ps = accfin[:, 136:264]
                    nc.tensor.transpose(agfT_ps, agf[:], id_t[:])
                    agfT = sm.tile([128, 128], f32, tag="agfTs", name="agfTs")
                    nc.scalar.activation(agfT[:], agfT_ps, ACT.Copy)
                    po = accfin[:, 264:392]
                    if not g.ZERO_BIAS:
                        nc.tensor.matmul(po, b_t["bo"][:], ones_t[:],
                                         start=True, stop=False)
                    nc.tensor.matmul(po, w_t["WoT"][:], agfT[:],
                                     start=g.ZERO_BIAS, stop=True)
                    oc = sm.tile([128, 128], f32, tag="oc", name="oc")
                    nc.scalar.activation(oc[:], po, ACT.Copy)
                    nc.sync.dma_start(outT[:, b * 128 : (b + 1) * 128], oc[:])

    nc.compile()
    return nc


# ---------------------------------------------------------------------------
# Entry point
# ---------------------------------------------------------------------------
N_NODES = 50000
N_CORES = 8

_CACHE = {}


def kernel(**inputs):
    from concourse.bass_utils import run_bass_kernel_spmd

    feats = np.asarray(inputs["feats"], np.float32)
    edge_index = np.asarray(inputs["edge_index"], np.int64)
    src = edge_index[:, 0]
    dst = edge_index[:, 1]

    zb = all(
        not np.any(np.asarray(inputs[k])) for k in ("bq", "bk", "bv")
    )
    SA, SB = compute_schedule(N_NODES, N_CORES, src, dst)
    g = Geom(N_NODES, N_CORES, SA, SB, zero_bias=zb)

    maps = host_prep(
        g, feats, edge_index,
        np.asarray(inputs["Wq"], np.float32), np.asarray(inputs["bq"], np.float32),
        np.asarray(inputs["Wk"], np.float32), np.asarray(inputs["bk"], np.float32),
        np.asarray(inputs["Wv"], np.float32), np.asarray(inputs["bv"], np.float32),
        np.asarray(inputs["Wo"], np.float32), np.asarray(inputs["bo"], np.float32),
    )

    key = (SA, SB, zb)
    if key not in _CACHE:
        _CACHE[key] = build_bass(g)
    nc = _CACHE[key]

    res = run_bass_kernel_spmd(nc, maps, list(range(N_CORES)))
    out = np.empty((N_NODES, g.D), np.float32)
    for c in range(N_CORES):
        out[c * g.NLOC : (c + 1) * g.NLOC][perms[c]] = (
            res.results[c]["outT"][:, : g.NLOC].T)
    return out
